# revision 10
# baseline (speedup 1.0000x reference)
"""Trainium2 Bass kernel for nn_MultiHeadHighLevelAllocator.

Math (reference):
    uav_embed = MLP_u(uav_feat)                     # (U=256, E=128)
    task_embed = MLP_t(task_feat)                   # (T=512, E=128)
    uq[h,u,:]  = uav_embed[u] + head_queries[h]     # (H=4, U, E)
    a[hu,k]    = uq[hu] @ Wu.T + fb0                # Wu = fw0[:, :E]
    b[t,k]     = task_embed[t] @ Wt.T               # Wt = fw0[:, E:]
    logits[hu,t] = sum_k fw1[k] * relu(a[hu,k] + b[t,k]) + fb1

Strategy (8 cores, shard T -> 64 t's per core, full HU on every core):
    - Prep matmuls on PE in feature-on-partition layout (host pre-transposes
      inputs), so a lands as a[k, hu] (2 k-tiles of (128, 1024)) and b as
      b[k, t_local] ((128, 64) per k-tile).
    - Fused bias+ReLU per (t, ktile) unit over the (128k, 1024hu) plane:
      ACT relu-with-bias, or DVE tensor_scalar add+max (2x fp32 mode).
      Units statically split across both engines to balance spans.
    - Contraction with fw1 on PE in fp16 (~3.6e-4 rel err end to end):
      lhsT = fw1 k-slice (128,1), rhs = R (128,512) x2 halves, M=1 outputs
      into PSUM partitions {0,32,64,96} (4 t's per round, col-tiled for
      concurrency), accumulated over the 2 k-tiles.
    - Eviction PSUM->SBUF adds fb1; strided-row DMA gathers the 4 t-rows.

Output per core: (64, 1024) fp32 [t_local, h*U+u]; host reassembles (H,U,T).
"""

import contextlib

import numpy as np

import concourse.bacc as bacc
import concourse.mybir as mybir
from concourse.tile import TileContext
from concourse.bass_utils import run_bass_kernel_spmd

U, T, H = 256, 512, 4
UAV_DIM, TASK_DIM, E, HID = 64, 32, 128, 256
HU = H * U                      # 1024
NCORES = 8
TL = T // NCORES                # 64 t's per core
NKT = HID // 128                # 2 k-tiles
NROUNDS = TL // 4               # 16 rounds of 4 t's

f32 = mybir.dt.float32
f16 = mybir.dt.float16
f32r = mybir.dt.float32r
AF = mybir.ActivationFunctionType
ALU = mybir.AluOpType
ET = mybir.EngineType

# Per-round unit assignment: 8 units = (j in 0..3) x (kt in 0..1).
# ACT unit ~1040ns, DVE unit ~594ns (fp32 2x mode) -> ACT:3, DVE:5.
ACT_UNITS = {(0, 0), (0, 1), (1, 0)}

IN_SPECS = [
    ("uavT", (UAV_DIM, U), f32),
    ("uw0T", (UAV_DIM, 128), f32),
    ("uw1T", (128, 128), f32),
    ("uw2T", (128, E), f32),
    ("ub0c", (128, 1), f32),
    ("ub1c", (128, 1), f32),
    ("hq2T", (E, H), f32),
    ("taskT", (TASK_DIM, TL), f32),
    ("tw0T", (TASK_DIM, 128), f32),
    ("tw1T", (128, 128), f32),
    ("tw2T", (128, E), f32),
    ("tb0c", (128, 1), f32),
    ("tb1c", (128, 1), f32),
    ("tb2c", (128, 1), f32),
    ("WuT", (E, HID), f32),
    ("WtT", (E, HID), f32),
    ("fb0c", (128, NKT), f32),
    ("fw1c", (128, NKT), f16),
    ("fb1s", (128, 1), f32),
]


def _emit_body(nc, d, pools, mult):
    singles, prep, ppsum, rpool, opool, fpsum = pools

    # ---- load inputs ----
    s = {}
    for name, shape, dt_ in IN_SPECS:
        s[name] = singles.tile(list(shape), dt_, name=name, tag=name)
        nc.sync.dma_start(out=s[name], in_=d[name][:])

    # ---- encoders + a/b prep ----
    uqT_s = singles.tile([E, HU], f32, name="uqT", tag="uqT")
    a_s = [singles.tile([128, HU], f32, tag=f"a{kt}", name=f"a{kt}")
           for kt in range(NKT)]
    a16_s = [singles.tile([128, HU], f16, tag=f"a16_{kt}", name=f"a16_{kt}")
             for kt in range(NKT)]
    b_s = [singles.tile([128, TL], f32, tag=f"b{kt}", name=f"b{kt}")
           for kt in range(NKT)]

    # uav + task encoders, chains interleaved so PE/ACT ping-pong.
    pe1 = ppsum.tile([128, U], f32, tag="pp", name="pe1")
    nc.tensor.matmul(pe1, s["uw0T"], s["uavT"], start=True, stop=True)
    pt1 = ppsum.tile([128, TL], f32, tag="pp", name="pt1")
    nc.tensor.matmul(pt1, s["tw0T"], s["taskT"], start=True, stop=True)
    h1 = prep.tile([128, U], f32, tag="pr", name="h1")
    nc.scalar.activation(h1, pe1, AF.Relu, bias=s["ub0c"][:, 0:1])
    s1 = prep.tile([128, TL], f32, tag="pr", name="s1")
    nc.scalar.activation(s1, pt1, AF.Relu, bias=s["tb0c"][:, 0:1])
    pe2 = ppsum.tile([128, U], f32, tag="pp", name="pe2")
    nc.tensor.matmul(pe2, s["uw1T"], h1, start=True, stop=True)
    pt2 = ppsum.tile([128, TL], f32, tag="pp", name="pt2")
    nc.tensor.matmul(pt2, s["tw1T"], s1, start=True, stop=True)
    h2 = prep.tile([128, U], f32, tag="pr", name="h2")
    nc.scalar.activation(h2, pe2, AF.Relu, bias=s["ub1c"][:, 0:1])
    s2 = prep.tile([128, TL], f32, tag="pr", name="s2")
    nc.scalar.activation(s2, pt2, AF.Relu, bias=s["tb1c"][:, 0:1])
    pe3 = ppsum.tile([E, U], f32, tag="pp", name="pe3")
    nc.tensor.matmul(pe3, s["uw2T"], h2, start=True, stop=True)
    pt3 = ppsum.tile([E, TL], f32, tag="pp", name="pt3")
    nc.tensor.matmul(pt3, s["tw2T"], s2, start=True, stop=True)
    # uqT[:, h-block] = uav_embedT + (head_queries[h] + ub2)   (ACT; f32r out)
    for h in range(H):
        nc.scalar.activation(
            uqT_s[:, h * U : (h + 1) * U], pe3, AF.Identity,
            bias=s["hq2T"][:, h : h + 1],
        )
    teT = prep.tile([E, TL], f32, tag="pr", name="teT")
    nc.scalar.activation(teT, pt3, AF.Identity, bias=s["tb2c"][:, 0:1])

    # b[kt] = (WtT slice).T @ teT  -> (128, TL)
    for kt in range(NKT):
        pb = ppsum.tile([128, TL], f32, tag="pp", name=f"pb{kt}")
        nc.tensor.matmul(pb, s["WtT"][:, kt * 128 : (kt + 1) * 128], teT,
                         start=True, stop=True)
        nc.vector.tensor_copy(out=b_s[kt], in_=pb)

    # a[kt] = (WuT slice).T @ uqT + fb0  -> (128, HU)
    for kt in range(NKT):
        for half in range(2):
            pa = ppsum.tile([128, 512], f32, tag="pp", name=f"pa{kt}{half}")
            nc.tensor.matmul(
                pa, s["WuT"][:, kt * 128 : (kt + 1) * 128],
                uqT_s[:, half * 512 : (half + 1) * 512],
                start=True, stop=True,
            )
            nc.scalar.activation(
                a_s[kt][:, half * 512 : (half + 1) * 512], pa,
                AF.Identity, bias=s["fb0c"][:, kt : kt + 1],
            )
            nc.vector.tensor_copy(
                out=a16_s[kt][:, half * 512 : (half + 1) * 512],
                in_=a_s[kt][:, half * 512 : (half + 1) * 512],
            )

    # ---- fusion rounds ----
    for rr in range(NROUNDS * mult):
        r = rr % NROUNDS
        ps_o = fpsum.tile([128, HU], f32, tag="ps_o", name=f"ps_o{rr}")
        act_units = {(0, 0), (0, 1)} if r % 8 in (0, 2, 4) else {(0, 0)}
        rt = {}
        for kt in range(NKT):
            for j in range(4):
                t = 4 * r + j
                Rt = rpool.tile([128, HU], f16, tag="R", name=f"R{rr}_{j}_{kt}")
                bias_ap = b_s[kt][:, t : t + 1]
                if (j, kt) in act_units:
                    nc.scalar.activation(Rt, a_s[kt], AF.Relu, bias=bias_ap)
                else:
                    nc.vector.tensor_scalar(
                        out=Rt, in0=a16_s[kt], scalar1=bias_ap,
                        scalar2=0.0, op0=ALU.add, op1=ALU.max,
                    )
                rt[(j, kt)] = Rt
        # contraction: interleave col groups for PE concurrency
        for kt in range(NKT):
            for half in range(2):
                for j in range(4):
                    nc.tensor.matmul(
                        ps_o[32 * j : 32 * j + 1,
                             half * 512 : (half + 1) * 512],
                        s["fw1c"][:, kt : kt + 1],
                        rt[(j, kt)][:, half * 512 : (half + 1) * 512],
                        start=(kt == 0), stop=(kt == NKT - 1),
                        tile_position=(0, 32 * j),
                    )
        # eviction (+fb1), alternating engine per round
        o_st = opool.tile([128, HU], f32, tag="o", name=f"o{rr}")
        nc.scalar.activation(o_st, ps_o, AF.Identity,
                             bias=s["fb1s"][:, 0:1])
        src = o_st.rearrange("(j i) n -> j i n", j=4)[:, 0, :]
        nc.sync.dma_start(out=d["out"][4 * r : 4 * r + 4, :], in_=src)


def _build_nc(mult=1, loop=None):
    nc = bacc.Bacc(None, target_bir_lowering=False)
    d = {}
    for name, shape, dt_ in IN_SPECS:
        d[name] = nc.dram_tensor(name, list(shape), dt_, kind="ExternalInput")
    d["out"] = nc.dram_tensor("out", [TL, HU], f32, kind="ExternalOutput")

    with TileContext(nc) as tc:
        with tc.tile_pool(name="singles", bufs=1) as singles, \
             tc.tile_pool(name="prep", bufs=2) as prep, \
             tc.tile_pool(name="ppsum", bufs=2, space="PSUM") as ppsum, \
             tc.tile_pool(name="rpool", bufs=24) as rpool, \
             tc.tile_pool(name="opool", bufs=3) as opool, \
             tc.tile_pool(name="fpsum", bufs=3, space="PSUM") as fpsum:
            pools = (singles, prep, ppsum, rpool, opool, fpsum)
            ctx = (tc.For_i(0, loop, 1,
                            hint_engines=(ET.PE, ET.Activation, ET.DVE))
                   if loop else contextlib.nullcontext())
            with ctx:
                _emit_body(nc, d, pools, mult)

    nc.finalize()
    return nc


_NC_CACHE = {}


def _get_nc(mult=1, loop=None):
    key = (mult, loop)
    if key not in _NC_CACHE:
        _NC_CACHE[key] = _build_nc(mult, loop)
    return _NC_CACHE[key]


def _prep_inputs(inputs):
    ct = np.ascontiguousarray
    f = np.float32
    uav_feat = inputs["uav_feat"].astype(f)
    task_feat = inputs["task_feat"].astype(f)
    base = {
        "uavT": ct(uav_feat.T),
        "uw0T": ct(inputs["uw0"].T.astype(f)),
        "uw1T": ct(inputs["uw1"].T.astype(f)),
        "uw2T": ct(inputs["uw2"].T.astype(f)),
        "ub0c": ct(inputs["ub0"].astype(f).reshape(128, 1)),
        "ub1c": ct(inputs["ub1"].astype(f).reshape(128, 1)),
        "hq2T": ct((inputs["head_queries"].astype(f)
                    + inputs["ub2"].astype(f)[None, :]).T),
        "tw0T": ct(inputs["tw0"].T.astype(f)),
        "tw1T": ct(inputs["tw1"].T.astype(f)),
        "tw2T": ct(inputs["tw2"].T.astype(f)),
        "tb0c": ct(inputs["tb0"].astype(f).reshape(128, 1)),
        "tb1c": ct(inputs["tb1"].astype(f).reshape(128, 1)),
        "tb2c": ct(inputs["tb2"].astype(f).reshape(128, 1)),
        "WuT": ct(inputs["fw0"][:, :E].T.astype(f)),
        "WtT": ct(inputs["fw0"][:, E:].T.astype(f)),
        "fb0c": ct(inputs["fb0"].astype(f).reshape(NKT, 128).T),
        "fw1c": ct(inputs["fw1"].reshape(NKT, 128).T.astype(np.float16)),
        "fb1s": ct(np.full((128, 1), float(inputs["fb1"][0]), dtype=f)),
    }
    taskT_full = ct(task_feat.T)
    in_maps = []
    for c in range(NCORES):
        m = dict(base)
        m["taskT"] = ct(taskT_full[:, c * TL : (c + 1) * TL])
        in_maps.append(m)
    return in_maps


def run(trace=False, **inputs):
    nc = _get_nc()
    in_maps = _prep_inputs(inputs)
    res = run_bass_kernel_spmd(nc, in_maps, list(range(NCORES)), trace=trace)
    big = np.concatenate([res.results[c]["out"] for c in range(NCORES)], axis=0)
    out = np.ascontiguousarray(big.T).reshape(H, U, T)
    return out, res


def kernel(**inputs):
    out, _ = run(**inputs)
    return out
